# revision 3
# baseline (speedup 1.0000x reference)
"""Trainium2 Bass kernel for nn_Evolution_4664334483942 (moe_routing).

Model: per-token relation-specific linear (MoE dispatch) feeding a packed
variable-length-sequence LSTM.

Strategy (data-parallel over sequences, 8 cores, no collectives):
  - Global batch b (0..1023) assigned to core b % 8.  Every core then holds
    128 sequences with lengths 128,127,...,1 (identical structure on every
    core), 8256 tokens each.
  - Host folds W_ih @ W_rel[r].T into per-relation fused weights so the MoE
    projection and the LSTM input projection collapse into ONE GEMM:
        gx[n] = x[n] @ Wfuse[rel_n].T + (W_ih b_rel[rel_n] + b_ih + b_hh)
  - Phase 1 (device): dense f32r GEMM over rel-sorted 128-token tiles,
    writing gx to DRAM.
  - Phase 2 (device): 128 sequential LSTM steps.  Each step gathers its
    gx rows via indirect DMA (per-core index table = data, so the SPMD
    instruction stream stays core-independent), feeds them into the gates
    PSUM via an identity matmul, accumulates h @ W_hh.T on top, applies
    sigmoid/tanh on ScalarE, c/h updates on VectorE, PE-transposes h for the
    next step, and streams h out to DRAM (contiguous rows).
"""

import numpy as np

import concourse.bass as bass
import concourse.mybir as mybir
import concourse.tile as tile
from concourse import bass_utils
from concourse.masks import make_identity
from concourse.vector_clock import ScopedClock

F32 = mybir.dt.float32
F32R = mybir.dt.float32r
I32 = mybir.dt.int32
AF = mybir.ActivationFunctionType

NCORES = 8

# Problem constants (hardcoded; kernel.py must be self-contained).
D = 512          # hidden dim
R = 8            # relations
T = 128          # max sequence length / LSTM steps
B = 1024         # global sequences
KD = D // 128    # contraction k-tiles
G = 4 * D        # gate width (2048)
NJB = G // 512   # psum banks for gates

NT_PER_REL = 12  # phase-1 128-token tiles reserved per relation (zero padded)
NT = R * NT_PER_REL

# Results of the last device run (test harness reads exec_time_ns from here).
LAST_RESULTS = None


# ---------------------------------------------------------------------------
# Walrus in this toolchain accepts only ONE sync-wait command per instruction;
# Tile's wait assignment can attach several.  Peel the extras onto same-engine
# NOPs placed immediately before the offending instruction.
# ---------------------------------------------------------------------------
def _split_waits_in_list(nc, insts, max_waits=1):
    out = []
    for inst in insts:
        si = inst.sync_info
        if si is not None and si.on_wait is not None and len(si.on_wait) > max_waits:
            waits = list(si.on_wait)
            for w in waits[max_waits:]:
                nop = mybir.InstNoOp(
                    name=nc.get_next_instruction_name(), ins=[], outs=[],
                )
                nop.engine = inst.engine
                nop.sync_info = mybir.SyncInfo(on_wait=[w], on_update=[])
                out.append(nop)
            inst.sync_info = mybir.SyncInfo(
                on_wait=waits[:max_waits], on_update=list(si.on_update or [])
            )
        out.append(inst)
    return out


class PatchedTileContext(tile.TileContext):
    def _lower_ordered_insts(self, ordered):
        for bb_name in list(ordered.keys()):
            ordered[bb_name] = _split_waits_in_list(self.nc, ordered[bb_name])
        super()._lower_ordered_insts(ordered)

    def _drain_and_barrier(self, tick_clock, wait_clock):
        nop_inst = self.nc.sync.nop()
        wait_clock.add_sem_waits(
            nop_inst.ins, ScopedClock({None: tick_clock.global_clock})
        )
        si = nop_inst.ins.sync_info
        if si is not None and si.on_wait and len(si.on_wait) > 1:
            waits = list(si.on_wait)
            nop_inst.ins.sync_info = mybir.SyncInfo(
                on_wait=[waits[0]], on_update=list(si.on_update or [])
            )
            for w in waits[1:]:
                extra = self.nc.sync.nop()
                extra.ins.sync_info = mybir.SyncInfo(on_wait=[w], on_update=[])
        self.nc.sync.drain()
        self.nc.all_engine_barrier()
        assert self.sems is not None
        popped = self.nc._tile_sem_poison_stack.pop()
        assert popped is self._sem_poison
        self.nc.clear_and_free_semaphores(list(self.sems.allocated().values()))
        self.nc.all_engine_barrier()


# ---------------------------------------------------------------------------
# Device program (core-independent instruction stream; per-core variation is
# carried entirely by input data: xt tile contents and the gather index table)
# ---------------------------------------------------------------------------
def build_program(nsteps=T, nt_per_rel=NT_PER_REL):
    ntiles = R * nt_per_rel
    nrows = ntiles * 128          # padded gx rows
    nloc = nsteps * (nsteps + 1) // 2

    nc = bass.Bass(target_bir_lowering=False, debug=False, trn_type="TRN2")

    xt = nc.dram_tensor("xt", [ntiles, 128, KD, 128], F32R, kind="ExternalInput").ap()
    wf = nc.dram_tensor("wf", [R, 128, KD, G], F32R, kind="ExternalInput").ap()
    wh = nc.dram_tensor("wh", [128, KD, G], F32R, kind="ExternalInput").ap()
    brep = nc.dram_tensor("brep", [R, 128, G], F32, kind="ExternalInput").ap()
    gidx = nc.dram_tensor("gidx", [128, nsteps], I32, kind="ExternalInput").ap()
    out = nc.dram_tensor("out", [nloc, D], F32, kind="ExternalOutput").ap()
    gx = nc.dram_tensor("gx", [nrows, G], F32R).ap()

    loc_bs = [nsteps - t for t in range(nsteps)]
    loc_off = np.concatenate([[0], np.cumsum(loc_bs)]).astype(int)

    with PatchedTileContext(nc) as tc:
        # ---------------- phase 1: gx = x @ Wfuse[r].T + bias -------------
        with tc.tile_pool(name="p1_xt", bufs=3) as xt_pool, \
             tc.tile_pool(name="p1_wf", bufs=2) as wf_pool, \
             tc.tile_pool(name="p1_bi", bufs=2) as bi_pool, \
             tc.tile_pool(name="p1_gx", bufs=3) as gxs_pool, \
             tc.tile_pool(name="p1_ps", bufs=2, space="PSUM") as ps1_pool:
            wf_sb = None
            bi_sb = None
            for i in range(ntiles):
                r = i // nt_per_rel
                if i % nt_per_rel == 0:
                    wf_sb = wf_pool.tile([128, KD, G], F32R, tag="wf_sb")
                    nc.sync.dma_start(wf_sb[:], wf[r])
                    bi_sb = bi_pool.tile([128, G], F32, tag="bi_sb")
                    nc.sync.dma_start(bi_sb[:], brep[r])
                xt_sb = xt_pool.tile([128, KD, 128], F32R, tag="xt_sb")
                nc.sync.dma_start(xt_sb[:], xt[i])
                ps = ps1_pool.tile([128, G], F32, tag="ps1")
                for k in range(KD):
                    for jb in range(NJB):
                        nc.tensor.matmul(
                            ps[:, jb * 512:(jb + 1) * 512],
                            xt_sb[:, k, :],
                            wf_sb[:, k, jb * 512:(jb + 1) * 512],
                            start=(k == 0),
                            stop=(k == KD - 1),
                        )
                gxs = gxs_pool.tile([128, G], F32R, tag="gxs")
                for jb in range(NJB):
                    sl = slice(jb * 512, (jb + 1) * 512)
                    nc.vector.tensor_add(gxs[:, sl], ps[:, sl], bi_sb[:, sl])
                nc.sync.dma_start(gx[i * 128:(i + 1) * 128, :], gxs[:])

        # ---------------- phase 2: LSTM over nsteps ------------------------
        with tc.tile_pool(name="p2_const", bufs=1) as const_pool, \
             tc.tile_pool(name="p2_gx", bufs=3) as gx_pool, \
             tc.tile_pool(name="p2_act", bufs=2) as act_pool, \
             tc.tile_pool(name="p2_st", bufs=1) as st_pool, \
             tc.tile_pool(name="p2_h", bufs=2) as h_pool, \
             tc.tile_pool(name="p2_ht", bufs=2) as ht_pool, \
             tc.tile_pool(name="p2_ps", bufs=1, space="PSUM") as ps2_pool, \
             tc.tile_pool(name="p2_tr", bufs=2, space="PSUM") as tr_pool:

            wh_sb = const_pool.tile([128, KD, G], F32R)
            nc.sync.dma_start(wh_sb[:], wh[:])
            idx_sb = const_pool.tile([128, nsteps], I32)
            nc.sync.dma_start(idx_sb[:], gidx[:])
            ident = const_pool.tile([128, 128], F32)
            make_identity(nc, ident[:])
            ident_r = const_pool.tile([128, 128], F32R)
            nc.vector.tensor_copy(ident_r[:], ident[:])

            c_sb = st_pool.tile([128, D], F32)
            tmp1 = st_pool.tile([128, D], F32)
            tmp2 = st_pool.tile([128, D], F32)

            ht_sb = None
            for t in range(nsteps):
                bs = nsteps - t
                # gather this step's gx rows into sequence order
                gxt = gx_pool.tile([128, G], F32R, tag="gxt")
                nc.gpsimd.indirect_dma_start(
                    out=gxt[:],
                    out_offset=None,
                    in_=gx[:],
                    in_offset=bass.IndirectOffsetOnAxis(
                        ap=idx_sb[:, t:t + 1], axis=0
                    ),
                )
                # gates = gx (via identity matmul) + h @ W_hh.T
                ps = ps2_pool.tile([128, G], F32, tag="ps2")
                for jb in range(NJB):
                    nc.tensor.matmul(
                        ps[:, jb * 512:(jb + 1) * 512],
                        ident_r[:],
                        gxt[:, jb * 512:(jb + 1) * 512],
                        start=True,
                        stop=(t == 0),
                    )
                if t > 0:
                    for k in range(KD):
                        for jb in range(NJB):
                            nc.tensor.matmul(
                                ps[:, jb * 512:(jb + 1) * 512],
                                ht_sb[:, k * 128:(k + 1) * 128],
                                wh_sb[:, k, jb * 512:(jb + 1) * 512],
                                start=False,
                                stop=(k == KD - 1),
                            )
                # activations: gates layout [i | f | g | o]
                sif = act_pool.tile([128, 2 * D], F32, tag="sif")
                nc.scalar.activation(sif[:, 0:D], ps[:, 0:D], AF.Sigmoid)
                nc.scalar.activation(sif[:, D:2 * D], ps[:, D:2 * D], AF.Sigmoid)
                tg = act_pool.tile([128, D], F32, tag="tg")
                nc.scalar.activation(tg[:], ps[:, 2 * D:3 * D], AF.Tanh)
                so = act_pool.tile([128, D], F32, tag="so")
                nc.scalar.activation(so[:], ps[:, 3 * D:4 * D], AF.Sigmoid)
                # c update
                if t == 0:
                    nc.vector.tensor_tensor(
                        c_sb[:], sif[:, 0:D], tg[:], mybir.AluOpType.mult
                    )
                else:
                    nc.vector.tensor_tensor(
                        tmp1[:], sif[:, D:2 * D], c_sb[:], mybir.AluOpType.mult
                    )
                    nc.vector.tensor_tensor(
                        tmp2[:], sif[:, 0:D], tg[:], mybir.AluOpType.mult
                    )
                    nc.vector.tensor_add(c_sb[:], tmp1[:], tmp2[:])
                tc_sb = act_pool.tile([128, D], F32, tag="tc_sb")
                nc.scalar.activation(tc_sb[:], c_sb[:], AF.Tanh)
                h_sb = h_pool.tile([128, D], F32, tag="h_sb")
                nc.vector.tensor_tensor(
                    h_sb[:], so[:], tc_sb[:], mybir.AluOpType.mult
                )
                # stream out this step's hidden states (packed rows)
                nc.sync.dma_start(
                    out[int(loc_off[t]):int(loc_off[t]) + bs, :], h_sb[:bs, :]
                )
                # transpose h for the next step's recurrent matmul
                if t < nsteps - 1:
                    trp = tr_pool.tile([128, D], F32, tag="trp")
                    for k in range(KD):
                        nc.tensor.transpose(
                            trp[:, k * 128:(k + 1) * 128],
                            h_sb[:, k * 128:(k + 1) * 128],
                            ident[:],
                        )
                    ht_sb = ht_pool.tile([128, D], F32R, tag="ht_sb")
                    nc.vector.tensor_copy(ht_sb[:], trp[:])
    return nc


# ---------------------------------------------------------------------------
# Host-side data marshaling
# ---------------------------------------------------------------------------
def _expected_layout():
    lengths = T - np.arange(B) // NCORES
    batch_sizes = np.array([(lengths > t).sum() for t in range(T)], dtype=np.int32)
    time_idx = np.concatenate(
        [np.full(bs, t, np.int32) for t, bs in enumerate(batch_sizes)]
    )
    batch_idx = np.concatenate(
        [np.arange(bs, dtype=np.int32) for bs in batch_sizes]
    )
    return batch_sizes, time_idx, batch_idx


def _numpy_reference(embed, W_rel, b_rel, W_ih, W_hh, b_ih, b_hh,
                     nodes, rels, time_idx, batch_idx, batch_sizes):
    """Pure-numpy fallback (only used if the packed layout differs from the
    hardcoded one)."""
    n_steps = int(batch_sizes.shape[0])
    max_bs = int(batch_sizes.max())
    x = embed[nodes]
    y = np.zeros_like(x)
    for r in range(W_rel.shape[0]):
        m = rels == r
        y[m] = x[m] @ W_rel[r].T + b_rel[r]
    d = x.shape[-1]
    xp = np.zeros((n_steps, max_bs, d), x.dtype)
    mask = np.zeros((n_steps, max_bs), bool)
    xp[time_idx, batch_idx] = y
    mask[time_idx, batch_idx] = True
    bias = b_ih + b_hh

    def sig(v):
        return 1.0 / (1.0 + np.exp(-v))

    h = np.zeros((max_bs, d), x.dtype)
    c = np.zeros((max_bs, d), x.dtype)
    hs = np.zeros((n_steps, max_bs, d), x.dtype)
    for t in range(n_steps):
        gates = xp[t] @ W_ih.T + h @ W_hh.T + bias
        i, f, g, o = np.split(gates, 4, axis=-1)
        c_new = sig(f) * c + sig(i) * np.tanh(g)
        h_new = sig(o) * np.tanh(c_new)
        m = mask[t][:, None]
        h = np.where(m, h_new, h)
        c = np.where(m, c_new, c)
        hs[t] = h
    return hs[time_idx, batch_idx]


def _prepare_host(inputs, nsteps=T, nt_per_rel=NT_PER_REL):
    """Build per-core device input dicts + the output unshard map."""
    embed = np.asarray(inputs["embed"], np.float32)
    W_rel = np.asarray(inputs["W_rel"], np.float32)
    b_rel = np.asarray(inputs["b_rel"], np.float32)
    W_ih = np.asarray(inputs["W_ih"], np.float32)
    W_hh = np.asarray(inputs["W_hh"], np.float32)
    b_ih = np.asarray(inputs["b_ih"], np.float32)
    b_hh = np.asarray(inputs["b_hh"], np.float32)
    nodes = np.asarray(inputs["nodes"])
    rels = np.asarray(inputs["rels"])

    ntiles = R * nt_per_rel
    nloc = nsteps * (nsteps + 1) // 2

    # fused weights & biases (float64 for accuracy, cast to f32)
    Wfuse = (W_ih.astype(np.float64) @ W_rel.astype(np.float64))
    Wfuse = Wfuse.astype(np.float32)            # [R, G, D]
    btot = (W_ih.astype(np.float64) @ b_rel.astype(np.float64).T).T \
        + (b_ih + b_hh).astype(np.float64)      # [R, G]
    btot = btot.astype(np.float32)

    # shared weight layouts
    wf_host = np.ascontiguousarray(
        Wfuse.transpose(0, 2, 1).reshape(R, KD, 128, G).transpose(0, 2, 1, 3)
    )                                            # [R, 128(dk), KD, G]
    wh_host = np.ascontiguousarray(
        W_hh.T.reshape(KD, 128, G).transpose(1, 0, 2)
    )                                            # [128(dk), KD, G]
    brep_host = np.ascontiguousarray(
        np.broadcast_to(btot[:, None, :], (R, 128, G))
    )

    # local token enumeration (identical structure for every core)
    t_arr = np.concatenate(
        [np.full(nsteps - t, t, np.int64) for t in range(nsteps)]
    )
    j_arr = np.concatenate(
        [np.arange(nsteps - t, dtype=np.int64) for t in range(nsteps)]
    )
    gbs = NCORES * (nsteps - np.arange(nsteps, dtype=np.int64))
    goff = np.concatenate([[0], np.cumsum(gbs)])

    in_maps = []
    for core in range(NCORES):
        grow = goff[t_arr] + NCORES * j_arr + core
        node_loc = nodes[grow]
        rel_loc = rels[grow].astype(np.int64)

        order = np.lexsort((j_arr, t_arr, rel_loc))
        cnt = np.bincount(rel_loc, minlength=R)
        if cnt.max() > nt_per_rel * 128:
            return None  # overflow -> caller falls back to numpy
        pbase = np.arange(R) * nt_per_rel * 128
        # padded row for each sorted token
        q = np.concatenate([np.arange(c) for c in cnt])
        prow_sorted = pbase[rel_loc[order]] + q
        prow = np.empty(nloc, np.int64)
        prow[order] = prow_sorted

        # gather index table: [128, nsteps]
        gidx_host = np.zeros((128, nsteps), np.int32)
        gidx_host[j_arr, t_arr] = prow

        # xt tiles
        Xp = np.zeros((ntiles * 128, D), np.float32)
        Xp[prow] = embed[node_loc]
        xt_host = np.ascontiguousarray(
            Xp.reshape(ntiles, 128, KD, 128).transpose(0, 3, 2, 1)
        )                                        # [NT, 128(dk), KD, 128(tok)]

        in_maps.append({
            "xt": xt_host,
            "wf": wf_host,
            "wh": wh_host,
            "brep": brep_host,
            "gidx": gidx_host,
        })

    unshard = {
        "t_arr": t_arr, "j_arr": j_arr, "goff": goff,
        "nloc": nloc,
    }
    return in_maps, unshard


def kernel(**inputs):
    global LAST_RESULTS
    import os

    # Verify the packed layout matches the hardcoded structure.
    bs_exp, ti_exp, bi_exp = _expected_layout()
    ok = (
        np.array_equal(np.asarray(inputs["batch_sizes"]), bs_exp)
        and np.array_equal(np.asarray(inputs["time_idx"]), ti_exp)
        and np.array_equal(np.asarray(inputs["batch_idx"]), bi_exp)
        and np.asarray(inputs["embed"]).shape == (50000, D)
    )
    if not ok:
        return _numpy_reference(**{k: np.asarray(v) for k, v in inputs.items()})

    prep = _prepare_host(inputs)
    if prep is None:
        return _numpy_reference(**{k: np.asarray(v) for k, v in inputs.items()})
    in_maps, unshard = prep

    nc = build_program()
    trace = bool(os.environ.get("KERNEL_TRACE"))
    res = bass_utils.run_bass_kernel_spmd(
        nc, in_maps, core_ids=list(range(NCORES)), trace=trace,
    )
    LAST_RESULTS = res

    t_arr = unshard["t_arr"]
    j_arr = unshard["j_arr"]
    goff = unshard["goff"]
    out_full = np.zeros((len(np.asarray(inputs["time_idx"])), D), np.float32)
    for core in range(NCORES):
        grow = goff[t_arr] + NCORES * j_arr + core
        out_full[grow] = res.results[core]["out"]
    return out_full


# revision 4
# speedup vs baseline: 1.1757x; 1.1757x over previous
"""Trainium2 Bass kernel for nn_Evolution_4664334483942 (moe_routing).

Model: per-token relation-specific linear (MoE dispatch) feeding a packed
variable-length-sequence LSTM.

Strategy (data-parallel over sequences, 8 cores, no collectives):
  - Global batch b (0..1023) assigned to core b % 8.  Every core then holds
    128 sequences with lengths 128,127,...,1 (identical structure on every
    core), 8256 tokens each.
  - Host folds W_ih @ W_rel[r].T into per-relation fused weights so the MoE
    projection and the LSTM input projection collapse into ONE GEMM:
        gx[n] = x[n] @ Wfuse[rel_n].T + (W_ih b_rel[rel_n] + b_ih + b_hh)
  - Phase 1 (device): dense f32r GEMM over rel-sorted 128-token tiles,
    writing gx to DRAM.
  - Phase 2 (device): 128 sequential LSTM steps.  Each step gathers its
    gx rows via indirect DMA (per-core index table = data, so the SPMD
    instruction stream stays core-independent), feeds them into the gates
    PSUM via an identity matmul, accumulates h @ W_hh.T on top, applies
    sigmoid/tanh on ScalarE, c/h updates on VectorE, PE-transposes h for the
    next step, and streams h out to DRAM (contiguous rows).
"""

import numpy as np

import concourse.bass as bass
import concourse.mybir as mybir
import concourse.tile as tile
from concourse import bass_utils
from concourse.masks import make_identity
from concourse.vector_clock import ScopedClock

F32 = mybir.dt.float32
F32R = mybir.dt.float32r
I32 = mybir.dt.int32
AF = mybir.ActivationFunctionType

NCORES = 8

# Problem constants (hardcoded; kernel.py must be self-contained).
D = 512          # hidden dim
R = 8            # relations
T = 128          # max sequence length / LSTM steps
B = 1024         # global sequences
KD = D // 128    # contraction k-tiles
G = 4 * D        # gate width (2048)
NJB = G // 512   # psum banks for gates

NT_PER_REL = 10  # phase-1 128-token tiles reserved per relation (zero padded)
NT = R * NT_PER_REL

# Results of the last device run (test harness reads exec_time_ns from here).
LAST_RESULTS = None


# ---------------------------------------------------------------------------
# Walrus in this toolchain accepts only ONE sync-wait command per instruction;
# Tile's wait assignment can attach several.  Peel the extras onto same-engine
# NOPs placed immediately before the offending instruction.
# ---------------------------------------------------------------------------
def _split_waits_in_list(nc, insts, max_waits=1):
    out = []
    for inst in insts:
        si = inst.sync_info
        if si is not None and si.on_wait is not None and len(si.on_wait) > max_waits:
            waits = list(si.on_wait)
            for w in waits[max_waits:]:
                nop = mybir.InstNoOp(
                    name=nc.get_next_instruction_name(), ins=[], outs=[],
                )
                nop.engine = inst.engine
                nop.sync_info = mybir.SyncInfo(on_wait=[w], on_update=[])
                out.append(nop)
            inst.sync_info = mybir.SyncInfo(
                on_wait=waits[:max_waits], on_update=list(si.on_update or [])
            )
        out.append(inst)
    return out


class PatchedTileContext(tile.TileContext):
    def _lower_ordered_insts(self, ordered):
        for bb_name in list(ordered.keys()):
            ordered[bb_name] = _split_waits_in_list(self.nc, ordered[bb_name])
        super()._lower_ordered_insts(ordered)

    def _drain_and_barrier(self, tick_clock, wait_clock):
        nop_inst = self.nc.sync.nop()
        wait_clock.add_sem_waits(
            nop_inst.ins, ScopedClock({None: tick_clock.global_clock})
        )
        si = nop_inst.ins.sync_info
        if si is not None and si.on_wait and len(si.on_wait) > 1:
            waits = list(si.on_wait)
            nop_inst.ins.sync_info = mybir.SyncInfo(
                on_wait=[waits[0]], on_update=list(si.on_update or [])
            )
            for w in waits[1:]:
                extra = self.nc.sync.nop()
                extra.ins.sync_info = mybir.SyncInfo(on_wait=[w], on_update=[])
        self.nc.sync.drain()
        self.nc.all_engine_barrier()
        assert self.sems is not None
        popped = self.nc._tile_sem_poison_stack.pop()
        assert popped is self._sem_poison
        self.nc.clear_and_free_semaphores(list(self.sems.allocated().values()))
        self.nc.all_engine_barrier()


# ---------------------------------------------------------------------------
# Device program (core-independent instruction stream; per-core variation is
# carried entirely by input data: xt tile contents and the gather index table)
# ---------------------------------------------------------------------------
def build_program(nsteps=T, nt_per_rel=NT_PER_REL):
    ntiles = R * nt_per_rel
    nrows = ntiles * 128          # padded gx rows
    nloc = nsteps * (nsteps + 1) // 2

    nc = bass.Bass(target_bir_lowering=False, debug=False, trn_type="TRN2")

    xt = nc.dram_tensor("xt", [ntiles, 128, KD, 128], F32R, kind="ExternalInput").ap()
    wf = nc.dram_tensor("wf", [R, 128, KD, G], F32R, kind="ExternalInput").ap()
    wh = nc.dram_tensor("wh", [128, KD, G], F32R, kind="ExternalInput").ap()
    brep = nc.dram_tensor("brep", [R, 128, G], F32, kind="ExternalInput").ap()
    gidx = nc.dram_tensor("gidx", [128, nsteps], I32, kind="ExternalInput").ap()
    out = nc.dram_tensor("out", [nloc, D], F32, kind="ExternalOutput").ap()
    gx = nc.dram_tensor("gx", [nrows, G], F32R).ap()

    loc_bs = [nsteps - t for t in range(nsteps)]
    loc_off = np.concatenate([[0], np.cumsum(loc_bs)]).astype(int)

    with PatchedTileContext(nc) as tc:
        # ---------------- phase 1: gx = x @ Wfuse[r].T + bias -------------
        with tc.tile_pool(name="p1_xt", bufs=3) as xt_pool, \
             tc.tile_pool(name="p1_wf", bufs=2) as wf_pool, \
             tc.tile_pool(name="p1_bi", bufs=2) as bi_pool, \
             tc.tile_pool(name="p1_gx", bufs=3) as gxs_pool, \
             tc.tile_pool(name="p1_ps", bufs=2, space="PSUM") as ps1_pool:
            wf_sb = None
            bi_sb = None
            for i in range(ntiles):
                r = i // nt_per_rel
                if i % nt_per_rel == 0:
                    wf_sb = wf_pool.tile([128, KD, G], F32R, tag="wf_sb")
                    nc.sync.dma_start(wf_sb[:], wf[r])
                    bi_sb = bi_pool.tile([128, G], F32, tag="bi_sb")
                    nc.sync.dma_start(bi_sb[:], brep[r])
                xt_sb = xt_pool.tile([128, KD, 128], F32R, tag="xt_sb")
                nc.sync.dma_start(xt_sb[:], xt[i])
                ps = ps1_pool.tile([128, G], F32, tag="ps1")
                for k in range(KD):
                    for jb in range(NJB):
                        nc.tensor.matmul(
                            ps[:, jb * 512:(jb + 1) * 512],
                            xt_sb[:, k, :],
                            wf_sb[:, k, jb * 512:(jb + 1) * 512],
                            start=(k == 0),
                            stop=(k == KD - 1),
                        )
                gxs = gxs_pool.tile([128, G], F32R, tag="gxs")
                for jb in range(NJB):
                    sl = slice(jb * 512, (jb + 1) * 512)
                    nc.vector.tensor_add(gxs[:, sl], ps[:, sl], bi_sb[:, sl])
                nc.sync.dma_start(gx[i * 128:(i + 1) * 128, :], gxs[:])

        # ---------------- phase 2: LSTM over nsteps ------------------------
        with tc.tile_pool(name="p2_const", bufs=1) as const_pool, \
             tc.tile_pool(name="p2_gx", bufs=3) as gx_pool, \
             tc.tile_pool(name="p2_act", bufs=2) as act_pool, \
             tc.tile_pool(name="p2_st", bufs=1) as st_pool, \
             tc.tile_pool(name="p2_h", bufs=2) as h_pool, \
             tc.tile_pool(name="p2_ht", bufs=2) as ht_pool, \
             tc.tile_pool(name="p2_ps", bufs=6, space="PSUM") as ps2_pool, \
             tc.tile_pool(name="p2_tr", bufs=2, space="PSUM") as tr_pool:

            wh_sb = const_pool.tile([128, KD, G], F32R)
            nc.sync.dma_start(wh_sb[:], wh[:])
            idx_sb = const_pool.tile([128, nsteps], I32)
            nc.sync.dma_start(idx_sb[:], gidx[:])
            ident = const_pool.tile([128, 128], F32)
            make_identity(nc, ident[:])

            c_sb = st_pool.tile([128, D], F32)
            tmp1 = st_pool.tile([128, D], F32)
            tmp2 = st_pool.tile([128, D], F32)

            ht_sb = None
            for t in range(nsteps):
                bs = nsteps - t
                # gather this step's gx rows into sequence order
                gxt = gx_pool.tile([128, G], F32, tag="gxt")
                nc.gpsimd.indirect_dma_start(
                    out=gxt[:],
                    out_offset=None,
                    in_=gx.bitcast(F32)[:],
                    in_offset=bass.IndirectOffsetOnAxis(
                        ap=idx_sb[:, t:t + 1], axis=0
                    ),
                )
                # gates = gx + h @ W_hh.T; bank-progressive so activations
                # overlap the remaining banks' matmuls
                gsb = act_pool.tile([128, G], F32, tag="gsb")
                for jb in range(NJB):
                    sl = slice(jb * 512, (jb + 1) * 512)
                    if t > 0:
                        psb = ps2_pool.tile([128, 512], F32, tag="ps2")
                        for k in range(KD):
                            nc.tensor.matmul(
                                psb[:],
                                ht_sb[:, k * 128:(k + 1) * 128],
                                wh_sb[:, k, sl],
                                start=(k == 0),
                                stop=(k == KD - 1),
                            )
                        nc.vector.tensor_add(gsb[:, sl], psb[:], gxt[:, sl])
                # activations: gates layout [i | f | g | o]
                gin = gxt if t == 0 else gsb
                sif = act_pool.tile([128, 2 * D], F32, tag="sif")
                nc.scalar.activation(sif[:, 0:D], gin[:, 0:D], AF.Sigmoid)
                nc.scalar.activation(sif[:, D:2 * D], gin[:, D:2 * D], AF.Sigmoid)
                tg = act_pool.tile([128, D], F32, tag="tg")
                nc.scalar.activation(tg[:], gin[:, 2 * D:3 * D], AF.Tanh)
                so = act_pool.tile([128, D], F32, tag="so")
                nc.scalar.activation(so[:], gin[:, 3 * D:4 * D], AF.Sigmoid)
                # c update
                if t == 0:
                    nc.vector.tensor_tensor(
                        c_sb[:], sif[:, 0:D], tg[:], mybir.AluOpType.mult
                    )
                else:
                    nc.vector.tensor_tensor(
                        tmp1[:], sif[:, D:2 * D], c_sb[:], mybir.AluOpType.mult
                    )
                    nc.vector.tensor_tensor(
                        tmp2[:], sif[:, 0:D], tg[:], mybir.AluOpType.mult
                    )
                    nc.vector.tensor_add(c_sb[:], tmp1[:], tmp2[:])
                tc_sb = act_pool.tile([128, D], F32, tag="tc_sb")
                nc.scalar.activation(tc_sb[:], c_sb[:], AF.Tanh)
                h_sb = h_pool.tile([128, D], F32, tag="h_sb")
                nc.vector.tensor_tensor(
                    h_sb[:], so[:], tc_sb[:], mybir.AluOpType.mult
                )
                # stream out this step's hidden states (packed rows)
                nc.sync.dma_start(
                    out[int(loc_off[t]):int(loc_off[t]) + bs, :], h_sb[:bs, :]
                )
                # transpose h for the next step's recurrent matmul
                if t < nsteps - 1:
                    trp = tr_pool.tile([128, D], F32, tag="trp")
                    for k in range(KD):
                        nc.tensor.transpose(
                            trp[:, k * 128:(k + 1) * 128],
                            h_sb[:, k * 128:(k + 1) * 128],
                            ident[:],
                        )
                    ht_sb = ht_pool.tile([128, D], F32R, tag="ht_sb")
                    nc.vector.tensor_copy(ht_sb[:], trp[:])
    return nc


# ---------------------------------------------------------------------------
# Host-side data marshaling
# ---------------------------------------------------------------------------
def _expected_layout():
    lengths = T - np.arange(B) // NCORES
    batch_sizes = np.array([(lengths > t).sum() for t in range(T)], dtype=np.int32)
    time_idx = np.concatenate(
        [np.full(bs, t, np.int32) for t, bs in enumerate(batch_sizes)]
    )
    batch_idx = np.concatenate(
        [np.arange(bs, dtype=np.int32) for bs in batch_sizes]
    )
    return batch_sizes, time_idx, batch_idx


def _numpy_reference(embed, W_rel, b_rel, W_ih, W_hh, b_ih, b_hh,
                     nodes, rels, time_idx, batch_idx, batch_sizes):
    """Pure-numpy fallback (only used if the packed layout differs from the
    hardcoded one)."""
    n_steps = int(batch_sizes.shape[0])
    max_bs = int(batch_sizes.max())
    x = embed[nodes]
    y = np.zeros_like(x)
    for r in range(W_rel.shape[0]):
        m = rels == r
        y[m] = x[m] @ W_rel[r].T + b_rel[r]
    d = x.shape[-1]
    xp = np.zeros((n_steps, max_bs, d), x.dtype)
    mask = np.zeros((n_steps, max_bs), bool)
    xp[time_idx, batch_idx] = y
    mask[time_idx, batch_idx] = True
    bias = b_ih + b_hh

    def sig(v):
        return 1.0 / (1.0 + np.exp(-v))

    h = np.zeros((max_bs, d), x.dtype)
    c = np.zeros((max_bs, d), x.dtype)
    hs = np.zeros((n_steps, max_bs, d), x.dtype)
    for t in range(n_steps):
        gates = xp[t] @ W_ih.T + h @ W_hh.T + bias
        i, f, g, o = np.split(gates, 4, axis=-1)
        c_new = sig(f) * c + sig(i) * np.tanh(g)
        h_new = sig(o) * np.tanh(c_new)
        m = mask[t][:, None]
        h = np.where(m, h_new, h)
        c = np.where(m, c_new, c)
        hs[t] = h
    return hs[time_idx, batch_idx]


def _prepare_host(inputs, nsteps=T, nt_per_rel=NT_PER_REL):
    """Build per-core device input dicts + the output unshard map."""
    embed = np.asarray(inputs["embed"], np.float32)
    W_rel = np.asarray(inputs["W_rel"], np.float32)
    b_rel = np.asarray(inputs["b_rel"], np.float32)
    W_ih = np.asarray(inputs["W_ih"], np.float32)
    W_hh = np.asarray(inputs["W_hh"], np.float32)
    b_ih = np.asarray(inputs["b_ih"], np.float32)
    b_hh = np.asarray(inputs["b_hh"], np.float32)
    nodes = np.asarray(inputs["nodes"])
    rels = np.asarray(inputs["rels"])

    ntiles = R * nt_per_rel
    nloc = nsteps * (nsteps + 1) // 2

    # fused weights & biases (float64 for accuracy, cast to f32)
    Wfuse = (W_ih.astype(np.float64) @ W_rel.astype(np.float64))
    Wfuse = Wfuse.astype(np.float32)            # [R, G, D]
    btot = (W_ih.astype(np.float64) @ b_rel.astype(np.float64).T).T \
        + (b_ih + b_hh).astype(np.float64)      # [R, G]
    btot = btot.astype(np.float32)

    # shared weight layouts
    wf_host = np.ascontiguousarray(
        Wfuse.transpose(0, 2, 1).reshape(R, KD, 128, G).transpose(0, 2, 1, 3)
    )                                            # [R, 128(dk), KD, G]
    wh_host = np.ascontiguousarray(
        W_hh.T.reshape(KD, 128, G).transpose(1, 0, 2)
    )                                            # [128(dk), KD, G]
    brep_host = np.ascontiguousarray(
        np.broadcast_to(btot[:, None, :], (R, 128, G))
    )

    # local token enumeration (identical structure for every core)
    t_arr = np.concatenate(
        [np.full(nsteps - t, t, np.int64) for t in range(nsteps)]
    )
    j_arr = np.concatenate(
        [np.arange(nsteps - t, dtype=np.int64) for t in range(nsteps)]
    )
    gbs = NCORES * (nsteps - np.arange(nsteps, dtype=np.int64))
    goff = np.concatenate([[0], np.cumsum(gbs)])

    in_maps = []
    for core in range(NCORES):
        grow = goff[t_arr] + NCORES * j_arr + core
        node_loc = nodes[grow]
        rel_loc = rels[grow].astype(np.int64)

        order = np.lexsort((j_arr, t_arr, rel_loc))
        cnt = np.bincount(rel_loc, minlength=R)
        if cnt.max() > nt_per_rel * 128:
            return None  # overflow -> caller falls back to numpy
        pbase = np.arange(R) * nt_per_rel * 128
        # padded row for each sorted token
        q = np.concatenate([np.arange(c) for c in cnt])
        prow_sorted = pbase[rel_loc[order]] + q
        prow = np.empty(nloc, np.int64)
        prow[order] = prow_sorted

        # gather index table: [128, nsteps]
        gidx_host = np.zeros((128, nsteps), np.int32)
        gidx_host[j_arr, t_arr] = prow

        # xt tiles
        Xp = np.zeros((ntiles * 128, D), np.float32)
        Xp[prow] = embed[node_loc]
        xt_host = np.ascontiguousarray(
            Xp.reshape(ntiles, 128, KD, 128).transpose(0, 3, 2, 1)
        )                                        # [NT, 128(dk), KD, 128(tok)]

        in_maps.append({
            "xt": xt_host,
            "wf": wf_host,
            "wh": wh_host,
            "brep": brep_host,
            "gidx": gidx_host,
        })

    unshard = {
        "t_arr": t_arr, "j_arr": j_arr, "goff": goff,
        "nloc": nloc,
    }
    return in_maps, unshard


def kernel(**inputs):
    global LAST_RESULTS
    import os

    # Verify the packed layout matches the hardcoded structure.
    bs_exp, ti_exp, bi_exp = _expected_layout()
    ok = (
        np.array_equal(np.asarray(inputs["batch_sizes"]), bs_exp)
        and np.array_equal(np.asarray(inputs["time_idx"]), ti_exp)
        and np.array_equal(np.asarray(inputs["batch_idx"]), bi_exp)
        and np.asarray(inputs["embed"]).shape == (50000, D)
    )
    if not ok:
        return _numpy_reference(**{k: np.asarray(v) for k, v in inputs.items()})

    prep = _prepare_host(inputs)
    if prep is None:
        return _numpy_reference(**{k: np.asarray(v) for k, v in inputs.items()})
    in_maps, unshard = prep

    nc = build_program()
    trace = bool(os.environ.get("KERNEL_TRACE"))
    res = bass_utils.run_bass_kernel_spmd(
        nc, in_maps, core_ids=list(range(NCORES)), trace=trace,
    )
    LAST_RESULTS = res

    t_arr = unshard["t_arr"]
    j_arr = unshard["j_arr"]
    goff = unshard["goff"]
    out_full = np.zeros((len(np.asarray(inputs["time_idx"])), D), np.float32)
    for core in range(NCORES):
        grow = goff[t_arr] + NCORES * j_arr + core
        out_full[grow] = res.results[core]["out"]
    return out_full


# revision 5
# speedup vs baseline: 1.2567x; 1.0689x over previous
"""Trainium2 Bass kernel for nn_Evolution_4664334483942 (moe_routing).

Model: per-token relation-specific linear (MoE dispatch) feeding a packed
variable-length-sequence LSTM.

Strategy (data-parallel over sequences, 8 cores, no collectives):
  - Global batch b (0..1023) assigned to core b % 8.  Every core then holds
    128 sequences with lengths 128,127,...,1 (identical structure on every
    core), 8256 tokens each.
  - Host folds W_ih @ W_rel[r].T into per-relation fused weights so the MoE
    projection and the LSTM input projection collapse into ONE GEMM:
        gx[n] = x[n] @ Wfuse[rel_n].T + (W_ih b_rel[rel_n] + b_ih + b_hh)
  - Phase 1 (device): dense f32r GEMM over rel-sorted 128-token tiles,
    writing gx to DRAM.
  - Phase 2 (device): 128 sequential LSTM steps.  Each step gathers its
    gx rows via indirect DMA (per-core index table = data, so the SPMD
    instruction stream stays core-independent), feeds them into the gates
    PSUM via an identity matmul, accumulates h @ W_hh.T on top, applies
    sigmoid/tanh on ScalarE, c/h updates on VectorE, PE-transposes h for the
    next step, and streams h out to DRAM (contiguous rows).
"""

import numpy as np

import concourse.bass as bass
import concourse.mybir as mybir
import concourse.tile as tile
from concourse import bass_utils
from concourse.masks import make_identity
from concourse.vector_clock import ScopedClock

F32 = mybir.dt.float32
F32R = mybir.dt.float32r
I32 = mybir.dt.int32
AF = mybir.ActivationFunctionType

NCORES = 8

# Problem constants (hardcoded; kernel.py must be self-contained).
D = 512          # hidden dim
R = 8            # relations
T = 128          # max sequence length / LSTM steps
B = 1024         # global sequences
KD = D // 128    # contraction k-tiles
G = 4 * D        # gate width (2048)
NJB = G // 512   # psum banks for gates

NT_PER_REL = 10  # phase-1 128-token tiles reserved per relation (zero padded)
NT = R * NT_PER_REL

# Results of the last device run (test harness reads exec_time_ns from here).
LAST_RESULTS = None


# ---------------------------------------------------------------------------
# Walrus in this toolchain accepts only ONE sync-wait command per instruction;
# Tile's wait assignment can attach several.  Peel the extras onto same-engine
# NOPs placed immediately before the offending instruction.
# ---------------------------------------------------------------------------
def _split_waits_in_list(nc, insts, max_waits=1):
    out = []
    for inst in insts:
        si = inst.sync_info
        if si is not None and si.on_wait is not None and len(si.on_wait) > max_waits:
            waits = list(si.on_wait)
            for w in waits[max_waits:]:
                nop = mybir.InstNoOp(
                    name=nc.get_next_instruction_name(), ins=[], outs=[],
                )
                nop.engine = inst.engine
                nop.sync_info = mybir.SyncInfo(on_wait=[w], on_update=[])
                out.append(nop)
            inst.sync_info = mybir.SyncInfo(
                on_wait=waits[:max_waits], on_update=list(si.on_update or [])
            )
        out.append(inst)
    return out


class PatchedTileContext(tile.TileContext):
    def _lower_ordered_insts(self, ordered):
        for bb_name in list(ordered.keys()):
            ordered[bb_name] = _split_waits_in_list(self.nc, ordered[bb_name])
        super()._lower_ordered_insts(ordered)

    def _drain_and_barrier(self, tick_clock, wait_clock):
        nop_inst = self.nc.sync.nop()
        wait_clock.add_sem_waits(
            nop_inst.ins, ScopedClock({None: tick_clock.global_clock})
        )
        si = nop_inst.ins.sync_info
        if si is not None and si.on_wait and len(si.on_wait) > 1:
            waits = list(si.on_wait)
            nop_inst.ins.sync_info = mybir.SyncInfo(
                on_wait=[waits[0]], on_update=list(si.on_update or [])
            )
            for w in waits[1:]:
                extra = self.nc.sync.nop()
                extra.ins.sync_info = mybir.SyncInfo(on_wait=[w], on_update=[])
        self.nc.sync.drain()
        self.nc.all_engine_barrier()
        assert self.sems is not None
        popped = self.nc._tile_sem_poison_stack.pop()
        assert popped is self._sem_poison
        self.nc.clear_and_free_semaphores(list(self.sems.allocated().values()))
        self.nc.all_engine_barrier()


# ---------------------------------------------------------------------------
# Device program (core-independent instruction stream; per-core variation is
# carried entirely by input data: xt tile contents and the gather index table)
# ---------------------------------------------------------------------------
def build_program(nsteps=T, nt_per_rel=NT_PER_REL):
    ntiles = R * nt_per_rel
    nrows = ntiles * 128          # padded gx rows
    nloc = nsteps * (nsteps + 1) // 2

    nc = bass.Bass(target_bir_lowering=False, debug=False, trn_type="TRN2")

    xt = nc.dram_tensor("xt", [ntiles, 128, KD, 128], F32R, kind="ExternalInput").ap()
    wf = nc.dram_tensor("wf", [R, 128, KD, G], F32R, kind="ExternalInput").ap()
    wh = nc.dram_tensor("wh", [128, KD, G], F32R, kind="ExternalInput").ap()
    brep = nc.dram_tensor("brep", [R, 128, G], F32, kind="ExternalInput").ap()
    gidx = nc.dram_tensor("gidx", [128, nsteps], I32, kind="ExternalInput").ap()
    out = nc.dram_tensor("out", [nloc, D], F32, kind="ExternalOutput").ap()
    gx = nc.dram_tensor("gx", [nrows, G], F32R).ap()

    loc_bs = [nsteps - t for t in range(nsteps)]
    loc_off = np.concatenate([[0], np.cumsum(loc_bs)]).astype(int)

    with PatchedTileContext(nc) as tc:
        # ---------------- phase 1: gx = x @ Wfuse[r].T + bias -------------
        with tc.tile_pool(name="p1_xt", bufs=3) as xt_pool, \
             tc.tile_pool(name="p1_wf", bufs=2) as wf_pool, \
             tc.tile_pool(name="p1_bi", bufs=2) as bi_pool, \
             tc.tile_pool(name="p1_gx", bufs=3) as gxs_pool, \
             tc.tile_pool(name="p1_ps", bufs=2, space="PSUM") as ps1_pool:
            wf_sb = None
            bi_sb = None
            for i in range(ntiles):
                r = i // nt_per_rel
                if i % nt_per_rel == 0:
                    wf_sb = wf_pool.tile([128, KD, G], F32R, tag="wf_sb")
                    nc.sync.dma_start(wf_sb[:], wf[r])
                    bi_sb = bi_pool.tile([128, G], F32, tag="bi_sb")
                    nc.sync.dma_start(bi_sb[:], brep[r])
                xt_sb = xt_pool.tile([128, KD, 128], F32R, tag="xt_sb")
                nc.sync.dma_start(xt_sb[:], xt[i])
                ps = ps1_pool.tile([128, G], F32, tag="ps1")
                for k in range(KD):
                    for jb in range(NJB):
                        nc.tensor.matmul(
                            ps[:, jb * 512:(jb + 1) * 512],
                            xt_sb[:, k, :],
                            wf_sb[:, k, jb * 512:(jb + 1) * 512],
                            start=(k == 0),
                            stop=(k == KD - 1),
                        )
                gxs = gxs_pool.tile([128, G], F32R, tag="gxs")
                for jb in range(NJB):
                    sl = slice(jb * 512, (jb + 1) * 512)
                    nc.vector.tensor_add(gxs[:, sl], ps[:, sl], bi_sb[:, sl])
                nc.sync.dma_start(gx[i * 128:(i + 1) * 128, :], gxs[:])

        # ---------------- phase 2: LSTM over nsteps ------------------------
        with tc.tile_pool(name="p2_const", bufs=1) as const_pool, \
             tc.tile_pool(name="p2_gx", bufs=3) as gx_pool, \
             tc.tile_pool(name="p2_act", bufs=2) as act_pool, \
             tc.tile_pool(name="p2_st", bufs=1) as st_pool, \
             tc.tile_pool(name="p2_h", bufs=2) as h_pool, \
             tc.tile_pool(name="p2_ht", bufs=2) as ht_pool, \
             tc.tile_pool(name="p2_ps", bufs=6, space="PSUM") as ps2_pool, \
             tc.tile_pool(name="p2_tr", bufs=2, space="PSUM") as tr_pool:

            wh_sb = const_pool.tile([128, KD, G], F32R)
            nc.sync.dma_start(wh_sb[:], wh[:])
            idx_sb = const_pool.tile([128, nsteps], I32)
            nc.sync.dma_start(idx_sb[:], gidx[:])
            ident = const_pool.tile([128, 128], F32)
            make_identity(nc, ident[:])
            ident_r = const_pool.tile([128, 128], F32R)
            nc.vector.tensor_copy(ident_r[:], ident[:])

            c_sb = st_pool.tile([128, D], F32)
            tmp1 = st_pool.tile([128, D], F32)
            tmp2 = st_pool.tile([128, D], F32)

            ht_sb = None
            for t in range(nsteps):
                bs = nsteps - t
                # gather this step's gx rows into sequence order
                gxt = gx_pool.tile([128, G], F32R, tag="gxt")
                nc.gpsimd.indirect_dma_start(
                    out=gxt[:],
                    out_offset=None,
                    in_=gx[:],
                    in_offset=bass.IndirectOffsetOnAxis(
                        ap=idx_sb[:, t:t + 1], axis=0
                    ),
                )
                # gates = gx (ident matmul) + h @ W_hh.T, bank-progressive;
                # process the g bank first so the c-update chain overlaps the
                # remaining banks' matmuls.  gates layout [i | f | g | o]
                banks = {}
                for jb in (2, 0, 1, 3):
                    sl = slice(jb * 512, (jb + 1) * 512)
                    psb = ps2_pool.tile([128, 512], F32, tag="ps2")
                    nc.tensor.matmul(
                        psb[:], ident_r[:], gxt[:, sl],
                        start=True, stop=(t == 0),
                    )
                    if t > 0:
                        for k in range(KD):
                            nc.tensor.matmul(
                                psb[:],
                                ht_sb[:, k * 128:(k + 1) * 128],
                                wh_sb[:, k, sl],
                                start=False,
                                stop=(k == KD - 1),
                            )
                    banks[jb] = psb
                sif = act_pool.tile([128, 2 * D], F32, tag="sif")
                tg = act_pool.tile([128, D], F32, tag="tg")
                so = act_pool.tile([128, D], F32, tag="so")
                nc.scalar.activation(tg[:], banks[2][:], AF.Tanh)
                nc.scalar.activation(sif[:, 0:D], banks[0][:], AF.Sigmoid)
                nc.scalar.activation(sif[:, D:2 * D], banks[1][:], AF.Sigmoid)
                nc.scalar.activation(so[:], banks[3][:], AF.Sigmoid)
                # c update
                if t == 0:
                    nc.vector.tensor_tensor(
                        c_sb[:], sif[:, 0:D], tg[:], mybir.AluOpType.mult
                    )
                else:
                    nc.gpsimd.tensor_tensor(
                        tmp2[:], sif[:, 0:D], tg[:], mybir.AluOpType.mult
                    )
                    nc.vector.tensor_tensor(
                        tmp1[:], sif[:, D:2 * D], c_sb[:], mybir.AluOpType.mult
                    )
                    nc.vector.tensor_add(c_sb[:], tmp1[:], tmp2[:])
                tc_sb = act_pool.tile([128, D], F32, tag="tc_sb")
                nc.scalar.activation(tc_sb[:], c_sb[:], AF.Tanh)
                h_sb = h_pool.tile([128, D], F32, tag="h_sb")
                nc.vector.tensor_tensor(
                    h_sb[:], so[:], tc_sb[:], mybir.AluOpType.mult
                )
                # stream out this step's hidden states (packed rows)
                nc.sync.dma_start(
                    out[int(loc_off[t]):int(loc_off[t]) + bs, :], h_sb[:bs, :]
                )
                # transpose h for the next step's recurrent matmul
                if t < nsteps - 1:
                    trp = tr_pool.tile([128, D], F32, tag="trp")
                    for k in range(KD):
                        nc.tensor.transpose(
                            trp[:, k * 128:(k + 1) * 128],
                            h_sb[:, k * 128:(k + 1) * 128],
                            ident[:],
                        )
                    ht_sb = ht_pool.tile([128, D], F32R, tag="ht_sb")
                    nc.vector.tensor_copy(ht_sb[:], trp[:])
    return nc


# ---------------------------------------------------------------------------
# Host-side data marshaling
# ---------------------------------------------------------------------------
def _expected_layout():
    lengths = T - np.arange(B) // NCORES
    batch_sizes = np.array([(lengths > t).sum() for t in range(T)], dtype=np.int32)
    time_idx = np.concatenate(
        [np.full(bs, t, np.int32) for t, bs in enumerate(batch_sizes)]
    )
    batch_idx = np.concatenate(
        [np.arange(bs, dtype=np.int32) for bs in batch_sizes]
    )
    return batch_sizes, time_idx, batch_idx


def _numpy_reference(embed, W_rel, b_rel, W_ih, W_hh, b_ih, b_hh,
                     nodes, rels, time_idx, batch_idx, batch_sizes):
    """Pure-numpy fallback (only used if the packed layout differs from the
    hardcoded one)."""
    n_steps = int(batch_sizes.shape[0])
    max_bs = int(batch_sizes.max())
    x = embed[nodes]
    y = np.zeros_like(x)
    for r in range(W_rel.shape[0]):
        m = rels == r
        y[m] = x[m] @ W_rel[r].T + b_rel[r]
    d = x.shape[-1]
    xp = np.zeros((n_steps, max_bs, d), x.dtype)
    mask = np.zeros((n_steps, max_bs), bool)
    xp[time_idx, batch_idx] = y
    mask[time_idx, batch_idx] = True
    bias = b_ih + b_hh

    def sig(v):
        return 1.0 / (1.0 + np.exp(-v))

    h = np.zeros((max_bs, d), x.dtype)
    c = np.zeros((max_bs, d), x.dtype)
    hs = np.zeros((n_steps, max_bs, d), x.dtype)
    for t in range(n_steps):
        gates = xp[t] @ W_ih.T + h @ W_hh.T + bias
        i, f, g, o = np.split(gates, 4, axis=-1)
        c_new = sig(f) * c + sig(i) * np.tanh(g)
        h_new = sig(o) * np.tanh(c_new)
        m = mask[t][:, None]
        h = np.where(m, h_new, h)
        c = np.where(m, c_new, c)
        hs[t] = h
    return hs[time_idx, batch_idx]


def _prepare_host(inputs, nsteps=T, nt_per_rel=NT_PER_REL):
    """Build per-core device input dicts + the output unshard map."""
    embed = np.asarray(inputs["embed"], np.float32)
    W_rel = np.asarray(inputs["W_rel"], np.float32)
    b_rel = np.asarray(inputs["b_rel"], np.float32)
    W_ih = np.asarray(inputs["W_ih"], np.float32)
    W_hh = np.asarray(inputs["W_hh"], np.float32)
    b_ih = np.asarray(inputs["b_ih"], np.float32)
    b_hh = np.asarray(inputs["b_hh"], np.float32)
    nodes = np.asarray(inputs["nodes"])
    rels = np.asarray(inputs["rels"])

    ntiles = R * nt_per_rel
    nloc = nsteps * (nsteps + 1) // 2

    # fused weights & biases (float64 for accuracy, cast to f32)
    Wfuse = (W_ih.astype(np.float64) @ W_rel.astype(np.float64))
    Wfuse = Wfuse.astype(np.float32)            # [R, G, D]
    btot = (W_ih.astype(np.float64) @ b_rel.astype(np.float64).T).T \
        + (b_ih + b_hh).astype(np.float64)      # [R, G]
    btot = btot.astype(np.float32)

    # shared weight layouts
    wf_host = np.ascontiguousarray(
        Wfuse.transpose(0, 2, 1).reshape(R, KD, 128, G).transpose(0, 2, 1, 3)
    )                                            # [R, 128(dk), KD, G]
    wh_host = np.ascontiguousarray(
        W_hh.T.reshape(KD, 128, G).transpose(1, 0, 2)
    )                                            # [128(dk), KD, G]
    brep_host = np.ascontiguousarray(
        np.broadcast_to(btot[:, None, :], (R, 128, G))
    )

    # local token enumeration (identical structure for every core)
    t_arr = np.concatenate(
        [np.full(nsteps - t, t, np.int64) for t in range(nsteps)]
    )
    j_arr = np.concatenate(
        [np.arange(nsteps - t, dtype=np.int64) for t in range(nsteps)]
    )
    gbs = NCORES * (nsteps - np.arange(nsteps, dtype=np.int64))
    goff = np.concatenate([[0], np.cumsum(gbs)])

    in_maps = []
    for core in range(NCORES):
        grow = goff[t_arr] + NCORES * j_arr + core
        node_loc = nodes[grow]
        rel_loc = rels[grow].astype(np.int64)

        order = np.lexsort((j_arr, t_arr, rel_loc))
        cnt = np.bincount(rel_loc, minlength=R)
        if cnt.max() > nt_per_rel * 128:
            return None  # overflow -> caller falls back to numpy
        pbase = np.arange(R) * nt_per_rel * 128
        # padded row for each sorted token
        q = np.concatenate([np.arange(c) for c in cnt])
        prow_sorted = pbase[rel_loc[order]] + q
        prow = np.empty(nloc, np.int64)
        prow[order] = prow_sorted

        # gather index table: [128, nsteps]
        gidx_host = np.zeros((128, nsteps), np.int32)
        gidx_host[j_arr, t_arr] = prow

        # xt tiles
        Xp = np.zeros((ntiles * 128, D), np.float32)
        Xp[prow] = embed[node_loc]
        xt_host = np.ascontiguousarray(
            Xp.reshape(ntiles, 128, KD, 128).transpose(0, 3, 2, 1)
        )                                        # [NT, 128(dk), KD, 128(tok)]

        in_maps.append({
            "xt": xt_host,
            "wf": wf_host,
            "wh": wh_host,
            "brep": brep_host,
            "gidx": gidx_host,
        })

    unshard = {
        "t_arr": t_arr, "j_arr": j_arr, "goff": goff,
        "nloc": nloc,
    }
    return in_maps, unshard


def kernel(**inputs):
    global LAST_RESULTS
    import os

    # Verify the packed layout matches the hardcoded structure.
    bs_exp, ti_exp, bi_exp = _expected_layout()
    ok = (
        np.array_equal(np.asarray(inputs["batch_sizes"]), bs_exp)
        and np.array_equal(np.asarray(inputs["time_idx"]), ti_exp)
        and np.array_equal(np.asarray(inputs["batch_idx"]), bi_exp)
        and np.asarray(inputs["embed"]).shape == (50000, D)
    )
    if not ok:
        return _numpy_reference(**{k: np.asarray(v) for k, v in inputs.items()})

    prep = _prepare_host(inputs)
    if prep is None:
        return _numpy_reference(**{k: np.asarray(v) for k, v in inputs.items()})
    in_maps, unshard = prep

    nc = build_program()
    trace = bool(os.environ.get("KERNEL_TRACE"))
    res = bass_utils.run_bass_kernel_spmd(
        nc, in_maps, core_ids=list(range(NCORES)), trace=trace,
    )
    LAST_RESULTS = res

    t_arr = unshard["t_arr"]
    j_arr = unshard["j_arr"]
    goff = unshard["goff"]
    out_full = np.zeros((len(np.asarray(inputs["time_idx"])), D), np.float32)
    for core in range(NCORES):
        grow = goff[t_arr] + NCORES * j_arr + core
        out_full[grow] = res.results[core]["out"]
    return out_full


# revision 7
# speedup vs baseline: 1.2752x; 1.0147x over previous
"""Trainium2 Bass kernel for nn_Evolution_4664334483942 (moe_routing).

Model: per-token relation-specific linear (MoE dispatch) feeding a packed
variable-length-sequence LSTM.

Strategy (data-parallel over sequences, 8 cores, no collectives):
  - Global batch b (0..1023) assigned to core b % 8.  Every core then holds
    128 sequences with lengths 128,127,...,1 (identical structure on every
    core), 8256 tokens each.
  - Host folds W_ih @ W_rel[r].T into per-relation fused weights so the MoE
    projection and the LSTM input projection collapse into ONE GEMM:
        gx[n] = x[n] @ Wfuse[rel_n].T + (W_ih b_rel[rel_n] + b_ih + b_hh)
  - Phase 1 (device): dense f32r GEMM over rel-sorted 128-token tiles,
    writing gx to DRAM.
  - Phase 2 (device): 128 sequential LSTM steps.  Each step gathers its
    gx rows via indirect DMA (per-core index table = data, so the SPMD
    instruction stream stays core-independent), feeds them into the gates
    PSUM via an identity matmul, accumulates h @ W_hh.T on top, applies
    sigmoid/tanh on ScalarE, c/h updates on VectorE, PE-transposes h for the
    next step, and streams h out to DRAM (contiguous rows).
"""

import numpy as np

import concourse.bass as bass
import concourse.mybir as mybir
import concourse.tile as tile
from concourse import bass_utils
from concourse.masks import make_identity
from concourse.vector_clock import ScopedClock

F32 = mybir.dt.float32
F32R = mybir.dt.float32r
I32 = mybir.dt.int32
AF = mybir.ActivationFunctionType

NCORES = 8

# Problem constants (hardcoded; kernel.py must be self-contained).
D = 512          # hidden dim
R = 8            # relations
T = 128          # max sequence length / LSTM steps
B = 1024         # global sequences
KD = D // 128    # contraction k-tiles
G = 4 * D        # gate width (2048)
NJB = G // 512   # psum banks for gates

NT_PER_REL = 10  # phase-1 128-token tiles reserved per relation (zero padded)
NT = R * NT_PER_REL

# Results of the last device run (test harness reads exec_time_ns from here).
LAST_RESULTS = None


# ---------------------------------------------------------------------------
# Walrus in this toolchain accepts only ONE sync-wait command per instruction;
# Tile's wait assignment can attach several.  Peel the extras onto same-engine
# NOPs placed immediately before the offending instruction.
# ---------------------------------------------------------------------------
def _split_waits_in_list(nc, insts, max_waits=1):
    out = []
    for inst in insts:
        si = inst.sync_info
        if si is not None and si.on_wait is not None and len(si.on_wait) > max_waits:
            waits = list(si.on_wait)
            for w in waits[max_waits:]:
                nop = mybir.InstNoOp(
                    name=nc.get_next_instruction_name(), ins=[], outs=[],
                )
                nop.engine = inst.engine
                nop.sync_info = mybir.SyncInfo(on_wait=[w], on_update=[])
                out.append(nop)
            inst.sync_info = mybir.SyncInfo(
                on_wait=waits[:max_waits], on_update=list(si.on_update or [])
            )
        out.append(inst)
    return out


class PatchedTileContext(tile.TileContext):
    def _lower_ordered_insts(self, ordered):
        for bb_name in list(ordered.keys()):
            ordered[bb_name] = _split_waits_in_list(self.nc, ordered[bb_name])
        super()._lower_ordered_insts(ordered)

    def _drain_and_barrier(self, tick_clock, wait_clock):
        nop_inst = self.nc.sync.nop()
        wait_clock.add_sem_waits(
            nop_inst.ins, ScopedClock({None: tick_clock.global_clock})
        )
        si = nop_inst.ins.sync_info
        if si is not None and si.on_wait and len(si.on_wait) > 1:
            waits = list(si.on_wait)
            nop_inst.ins.sync_info = mybir.SyncInfo(
                on_wait=[waits[0]], on_update=list(si.on_update or [])
            )
            for w in waits[1:]:
                extra = self.nc.sync.nop()
                extra.ins.sync_info = mybir.SyncInfo(on_wait=[w], on_update=[])
        self.nc.sync.drain()
        self.nc.all_engine_barrier()
        assert self.sems is not None
        popped = self.nc._tile_sem_poison_stack.pop()
        assert popped is self._sem_poison
        self.nc.clear_and_free_semaphores(list(self.sems.allocated().values()))
        self.nc.all_engine_barrier()


# ---------------------------------------------------------------------------
# Device program (core-independent instruction stream; per-core variation is
# carried entirely by input data: xt tile contents and the gather index table)
# ---------------------------------------------------------------------------
def build_program(nsteps=T, nt_per_rel=NT_PER_REL):
    ntiles = R * nt_per_rel
    nrows = ntiles * 128          # padded gx rows
    nloc = nsteps * (nsteps + 1) // 2

    nc = bass.Bass(target_bir_lowering=False, debug=False, trn_type="TRN2")

    xt = nc.dram_tensor("xt", [ntiles, 128, KD, 128], F32R, kind="ExternalInput").ap()
    wf = nc.dram_tensor("wf", [R, 128, KD, G], F32R, kind="ExternalInput").ap()
    wh = nc.dram_tensor("wh", [128, KD, G], F32R, kind="ExternalInput").ap()
    brep = nc.dram_tensor("brep", [R, 128, G], F32, kind="ExternalInput").ap()
    gidx = nc.dram_tensor("gidx", [128, nsteps], I32, kind="ExternalInput").ap()
    out = nc.dram_tensor("out", [nloc, D], F32, kind="ExternalOutput").ap()
    gx = nc.dram_tensor("gx", [nrows, G], F32R).ap()

    loc_bs = [nsteps - t for t in range(nsteps)]
    loc_off = np.concatenate([[0], np.cumsum(loc_bs)]).astype(int)

    with PatchedTileContext(nc) as tc:
        # ---------------- phase 1: gx = x @ Wfuse[r].T + bias -------------
        with tc.tile_pool(name="p1_xt", bufs=3) as xt_pool, \
             tc.tile_pool(name="p1_wf", bufs=2) as wf_pool, \
             tc.tile_pool(name="p1_bi", bufs=2) as bi_pool, \
             tc.tile_pool(name="p1_gx", bufs=3) as gxs_pool, \
             tc.tile_pool(name="p1_ps", bufs=2, space="PSUM") as ps1_pool:
            wf_sb = None
            bi_sb = None
            for i in range(ntiles):
                r = i // nt_per_rel
                if i % nt_per_rel == 0:
                    wf_sb = wf_pool.tile([128, KD, G], F32R, tag="wf_sb")
                    nc.sync.dma_start(wf_sb[:], wf[r])
                    bi_sb = bi_pool.tile([128, G], F32, tag="bi_sb")
                    nc.sync.dma_start(bi_sb[:], brep[r])
                xt_sb = xt_pool.tile([128, KD, 128], F32R, tag="xt_sb")
                nc.sync.dma_start(xt_sb[:], xt[i])
                ps = ps1_pool.tile([128, G], F32, tag="ps1")
                for k in range(KD):
                    for jb in range(NJB):
                        nc.tensor.matmul(
                            ps[:, jb * 512:(jb + 1) * 512],
                            xt_sb[:, k, :],
                            wf_sb[:, k, jb * 512:(jb + 1) * 512],
                            start=(k == 0),
                            stop=(k == KD - 1),
                        )
                gxs = gxs_pool.tile([128, G], F32R, tag="gxs")
                for jb in range(NJB):
                    sl = slice(jb * 512, (jb + 1) * 512)
                    nc.vector.tensor_add(gxs[:, sl], ps[:, sl], bi_sb[:, sl])
                nc.sync.dma_start(gx[i * 128:(i + 1) * 128, :], gxs[:])

        # ---------------- phase 2: LSTM over nsteps ------------------------
        with tc.tile_pool(name="p2_const", bufs=1) as const_pool, \
             tc.tile_pool(name="p2_gx", bufs=3) as gx_pool, \
             tc.tile_pool(name="p2_act", bufs=2) as act_pool, \
             tc.tile_pool(name="p2_st", bufs=1) as st_pool, \
             tc.tile_pool(name="p2_h", bufs=2) as h_pool, \
             tc.tile_pool(name="p2_ht", bufs=2) as ht_pool, \
             tc.tile_pool(name="p2_ps", bufs=5, space="PSUM") as ps2_pool, \
             tc.tile_pool(name="p2_tr", bufs=2, space="PSUM") as tr_pool:

            wh_sb = const_pool.tile([128, KD, G], F32R)
            nc.sync.dma_start(wh_sb[:], wh[:])
            idx_sb = const_pool.tile([128, nsteps], I32)
            nc.sync.dma_start(idx_sb[:], gidx[:])
            ident = const_pool.tile([128, 128], F32)
            make_identity(nc, ident[:])
            ident_r = const_pool.tile([128, 128], F32R)
            nc.vector.tensor_copy(ident_r[:], ident[:])

            c_sb = st_pool.tile([128, D], F32)
            tmp1 = st_pool.tile([128, D], F32)
            tmp2 = st_pool.tile([128, D], F32)

            ht_sb = None
            gxt_tiles = {}
            banks = {}

            def emit_gather(t):
                gxt = gx_pool.tile([128, G], F32R, tag="gxt")
                nc.gpsimd.indirect_dma_start(
                    out=gxt[:],
                    out_offset=None,
                    in_=gx[:],
                    in_offset=bass.IndirectOffsetOnAxis(
                        ap=idx_sb[:, t:t + 1], axis=0
                    ),
                )
                gxt_tiles[t] = gxt

            def emit_ident(t, jb):
                # first write of bank jb for step t: gates <- gx rows
                psb = ps2_pool.tile([128, 512], F32, tag="ps2")
                nc.tensor.matmul(
                    psb[:], ident_r[:],
                    gxt_tiles[t][:, jb * 512:(jb + 1) * 512],
                    start=True, stop=(t == 0),
                )
                banks[(t, jb)] = psb

            # bank processing order: g first so the c-chain overlaps later banks
            BORD = (2, 0, 1, 3)
            emit_gather(0)
            emit_gather(1)
            for jb in BORD:
                emit_ident(0, jb)
            for t in range(nsteps):
                bs = nsteps - t
                if t + 2 < nsteps:
                    emit_gather(t + 2)
                # recurrent accumulation (h @ W_hh.T) on top of the gx banks
                if t > 0:
                    for jb in BORD:
                        sl = slice(jb * 512, (jb + 1) * 512)
                        psb = banks[(t, jb)]
                        for k in range(KD):
                            nc.tensor.matmul(
                                psb[:],
                                ht_sb[:, k * 128:(k + 1) * 128],
                                wh_sb[:, k, sl],
                                start=False,
                                stop=(k == KD - 1),
                            )
                # activations (gates layout [i | f | g | o]); after each bank
                # is consumed, emit the next step's ident matmul into the slot
                # rotation so the PE stays busy through the step tail
                sif = act_pool.tile([128, 2 * D], F32, tag="sif")
                tg = act_pool.tile([128, D], F32, tag="tg")
                so = act_pool.tile([128, D], F32, tag="so")
                act_of = {
                    2: (tg[:], AF.Tanh),
                    0: (sif[:, 0:D], AF.Sigmoid),
                    1: (sif[:, D:2 * D], AF.Sigmoid),
                    3: (so[:], AF.Sigmoid),
                }
                for jb in BORD:
                    dst, fn = act_of[jb]
                    nc.scalar.activation(dst, banks.pop((t, jb))[:], fn)
                    if t + 1 < nsteps:
                        emit_ident(t + 1, jb)
                # c update
                if t == 0:
                    nc.vector.tensor_tensor(
                        c_sb[:], sif[:, 0:D], tg[:], mybir.AluOpType.mult
                    )
                else:
                    nc.gpsimd.tensor_tensor(
                        tmp2[:], sif[:, 0:D], tg[:], mybir.AluOpType.mult
                    )
                    nc.vector.tensor_tensor(
                        tmp1[:], sif[:, D:2 * D], c_sb[:], mybir.AluOpType.mult
                    )
                    nc.vector.tensor_add(c_sb[:], tmp1[:], tmp2[:])
                tc_sb = act_pool.tile([128, D], F32, tag="tc_sb")
                nc.scalar.activation(tc_sb[:], c_sb[:], AF.Tanh)
                h_sb = h_pool.tile([128, D], F32, tag="h_sb")
                nc.vector.tensor_tensor(
                    h_sb[:], so[:], tc_sb[:], mybir.AluOpType.mult
                )
                # stream out this step's hidden states (packed rows)
                nc.sync.dma_start(
                    out[int(loc_off[t]):int(loc_off[t]) + bs, :], h_sb[:bs, :]
                )
                # transpose h for the next step's recurrent matmul
                if t < nsteps - 1:
                    trp = tr_pool.tile([128, D], F32, tag="trp")
                    for k in range(KD):
                        nc.tensor.transpose(
                            trp[:, k * 128:(k + 1) * 128],
                            h_sb[:, k * 128:(k + 1) * 128],
                            ident[:],
                        )
                    ht_sb = ht_pool.tile([128, D], F32R, tag="ht_sb")
                    nc.vector.tensor_copy(ht_sb[:], trp[:])
    return nc


# ---------------------------------------------------------------------------
# Host-side data marshaling
# ---------------------------------------------------------------------------
def _expected_layout():
    lengths = T - np.arange(B) // NCORES
    batch_sizes = np.array([(lengths > t).sum() for t in range(T)], dtype=np.int32)
    time_idx = np.concatenate(
        [np.full(bs, t, np.int32) for t, bs in enumerate(batch_sizes)]
    )
    batch_idx = np.concatenate(
        [np.arange(bs, dtype=np.int32) for bs in batch_sizes]
    )
    return batch_sizes, time_idx, batch_idx


def _numpy_reference(embed, W_rel, b_rel, W_ih, W_hh, b_ih, b_hh,
                     nodes, rels, time_idx, batch_idx, batch_sizes):
    """Pure-numpy fallback (only used if the packed layout differs from the
    hardcoded one)."""
    n_steps = int(batch_sizes.shape[0])
    max_bs = int(batch_sizes.max())
    x = embed[nodes]
    y = np.zeros_like(x)
    for r in range(W_rel.shape[0]):
        m = rels == r
        y[m] = x[m] @ W_rel[r].T + b_rel[r]
    d = x.shape[-1]
    xp = np.zeros((n_steps, max_bs, d), x.dtype)
    mask = np.zeros((n_steps, max_bs), bool)
    xp[time_idx, batch_idx] = y
    mask[time_idx, batch_idx] = True
    bias = b_ih + b_hh

    def sig(v):
        return 1.0 / (1.0 + np.exp(-v))

    h = np.zeros((max_bs, d), x.dtype)
    c = np.zeros((max_bs, d), x.dtype)
    hs = np.zeros((n_steps, max_bs, d), x.dtype)
    for t in range(n_steps):
        gates = xp[t] @ W_ih.T + h @ W_hh.T + bias
        i, f, g, o = np.split(gates, 4, axis=-1)
        c_new = sig(f) * c + sig(i) * np.tanh(g)
        h_new = sig(o) * np.tanh(c_new)
        m = mask[t][:, None]
        h = np.where(m, h_new, h)
        c = np.where(m, c_new, c)
        hs[t] = h
    return hs[time_idx, batch_idx]


def _prepare_host(inputs, nsteps=T, nt_per_rel=NT_PER_REL):
    """Build per-core device input dicts + the output unshard map."""
    embed = np.asarray(inputs["embed"], np.float32)
    W_rel = np.asarray(inputs["W_rel"], np.float32)
    b_rel = np.asarray(inputs["b_rel"], np.float32)
    W_ih = np.asarray(inputs["W_ih"], np.float32)
    W_hh = np.asarray(inputs["W_hh"], np.float32)
    b_ih = np.asarray(inputs["b_ih"], np.float32)
    b_hh = np.asarray(inputs["b_hh"], np.float32)
    nodes = np.asarray(inputs["nodes"])
    rels = np.asarray(inputs["rels"])

    ntiles = R * nt_per_rel
    nloc = nsteps * (nsteps + 1) // 2

    # fused weights & biases (float64 for accuracy, cast to f32)
    Wfuse = (W_ih.astype(np.float64) @ W_rel.astype(np.float64))
    Wfuse = Wfuse.astype(np.float32)            # [R, G, D]
    btot = (W_ih.astype(np.float64) @ b_rel.astype(np.float64).T).T \
        + (b_ih + b_hh).astype(np.float64)      # [R, G]
    btot = btot.astype(np.float32)

    # shared weight layouts
    wf_host = np.ascontiguousarray(
        Wfuse.transpose(0, 2, 1).reshape(R, KD, 128, G).transpose(0, 2, 1, 3)
    )                                            # [R, 128(dk), KD, G]
    wh_host = np.ascontiguousarray(
        W_hh.T.reshape(KD, 128, G).transpose(1, 0, 2)
    )                                            # [128(dk), KD, G]
    brep_host = np.ascontiguousarray(
        np.broadcast_to(btot[:, None, :], (R, 128, G))
    )

    # local token enumeration (identical structure for every core)
    t_arr = np.concatenate(
        [np.full(nsteps - t, t, np.int64) for t in range(nsteps)]
    )
    j_arr = np.concatenate(
        [np.arange(nsteps - t, dtype=np.int64) for t in range(nsteps)]
    )
    gbs = NCORES * (nsteps - np.arange(nsteps, dtype=np.int64))
    goff = np.concatenate([[0], np.cumsum(gbs)])

    in_maps = []
    for core in range(NCORES):
        grow = goff[t_arr] + NCORES * j_arr + core
        node_loc = nodes[grow]
        rel_loc = rels[grow].astype(np.int64)

        order = np.lexsort((j_arr, t_arr, rel_loc))
        cnt = np.bincount(rel_loc, minlength=R)
        if cnt.max() > nt_per_rel * 128:
            return None  # overflow -> caller falls back to numpy
        pbase = np.arange(R) * nt_per_rel * 128
        # padded row for each sorted token
        q = np.concatenate([np.arange(c) for c in cnt])
        prow_sorted = pbase[rel_loc[order]] + q
        prow = np.empty(nloc, np.int64)
        prow[order] = prow_sorted

        # gather index table: [128, nsteps]
        gidx_host = np.zeros((128, nsteps), np.int32)
        gidx_host[j_arr, t_arr] = prow

        # xt tiles
        Xp = np.zeros((ntiles * 128, D), np.float32)
        Xp[prow] = embed[node_loc]
        xt_host = np.ascontiguousarray(
            Xp.reshape(ntiles, 128, KD, 128).transpose(0, 3, 2, 1)
        )                                        # [NT, 128(dk), KD, 128(tok)]

        in_maps.append({
            "xt": xt_host,
            "wf": wf_host,
            "wh": wh_host,
            "brep": brep_host,
            "gidx": gidx_host,
        })

    unshard = {
        "t_arr": t_arr, "j_arr": j_arr, "goff": goff,
        "nloc": nloc,
    }
    return in_maps, unshard


def kernel(**inputs):
    global LAST_RESULTS
    import os

    # Verify the packed layout matches the hardcoded structure.
    bs_exp, ti_exp, bi_exp = _expected_layout()
    ok = (
        np.array_equal(np.asarray(inputs["batch_sizes"]), bs_exp)
        and np.array_equal(np.asarray(inputs["time_idx"]), ti_exp)
        and np.array_equal(np.asarray(inputs["batch_idx"]), bi_exp)
        and np.asarray(inputs["embed"]).shape == (50000, D)
    )
    if not ok:
        return _numpy_reference(**{k: np.asarray(v) for k, v in inputs.items()})

    prep = _prepare_host(inputs)
    if prep is None:
        return _numpy_reference(**{k: np.asarray(v) for k, v in inputs.items()})
    in_maps, unshard = prep

    nc = build_program()
    trace = bool(os.environ.get("KERNEL_TRACE"))
    res = bass_utils.run_bass_kernel_spmd(
        nc, in_maps, core_ids=list(range(NCORES)), trace=trace,
    )
    LAST_RESULTS = res

    t_arr = unshard["t_arr"]
    j_arr = unshard["j_arr"]
    goff = unshard["goff"]
    out_full = np.zeros((len(np.asarray(inputs["time_idx"])), D), np.float32)
    for core in range(NCORES):
        grow = goff[t_arr] + NCORES * j_arr + core
        out_full[grow] = res.results[core]["out"]
    return out_full
